# revision 31
# baseline (speedup 1.0000x reference)
"""Trainium2 Bass kernel for AIMQuantizerForVJEPA (residual VQ, 3 levels).

Math (forward pass):
  r0 = z @ W_in + b_in                      [BN, 256]
  per level l: score_l = r_l @ emb_l.T ; idx_l = argmax(score_l)
  (equivalent to argmin of squared distance since ||e_k||^2 == 1 up to ~1e-7)
  z_q_l = emb_l[idx_l] ; r_{l+1} = r_l - z_q_l
  z_q_out = (z_q_0+z_q_1+z_q_2) @ W_out + b_out
  vq_loss = 0.25/(3*BN*P) * (S1+S2+S3),  S_{l+1} = sum_t ||r_{l+1},t||^2
  with the telescoping identity S_{l+1} = S_l - 2*sum_t max_t + sum_t ||e_sel||^2
  evaluated on the host from small device-side stashes.

Precision: all argmax-relevant matmuls run as bf16 3-term splits
(ah@bh + ah@bl + al@bh with x = xh + xl, xh=bf16(x), xl=bf16(x-xh));
operand error ~2^-17, products accumulate in fp32 -> fp32-grade scores at
3 cycles/row instead of fp32's 4, with FWL-fast weight loads.
Residuals for levels 1/2 are never materialized: scores are corrected with
host-precomputed Gram matrices (score_1 = r0@emb1.T - G01[idx0], ...), the
row gathers being onehot matmuls. Onehot transposes ride the DMA xbar
(bf16-only path), not the PE. The output-side matmuls (onehot@emb, zq@W_out)
use float32r / bf16 pairs; their error does not feed any argmax.

Sharding: data-parallel over batch, 2 batches (4096 tokens) per core, 8 cores.
Device layout is channel-on-partitions; the host pre/post-transposes z and
z_q_out.
"""

import sys

for _p in ("/opt/trn_rl_repo", "/opt/pypackages"):
    if _p not in sys.path:
        sys.path.insert(0, _p)

import ml_dtypes
import numpy as np

import concourse.bass as bass
import concourse.mybir as mybir
import concourse.tile as tile
from concourse.bass import ts
from concourse.bass_utils import run_bass_kernel_spmd

dt = mybir.dt
F32 = dt.float32
F32R = dt.float32r
BF16 = dt.bfloat16
AF = mybir.ActivationFunctionType
ALU = mybir.AluOpType
AX = mybir.AxisListType
NPBF = ml_dtypes.bfloat16

B, N, D, P = 16, 2048, 1408, 256
KS = (64, 128, 256)
COMMIT = 0.25
NCORES = 8
TOK = B * N // NCORES        # 4096 tokens per core
NG = 8                       # groups per core
TG = TOK // NG               # 512 tokens per group
NT = TG // 128               # 4 tiles of 128 tokens per group
ND = D // 128                # 11 d-tiles
# level column ranges inside the 512-wide score panel (128-aligned windows so
# the DMA-xbar transpose of each window lands as one [128,128] block)
LOFF = (64, 128, 256)
LEND = (128, 256, 512)


def _legalize_sync_waits(nc, limit=1):
    """This walrus build allows only one sync-wait per instruction; move
    excess waits onto dedicated NoOps just before their instruction."""
    import bass_rust

    n = 0
    for bb in nc.main_func.blocks:
        insts = list(bb.instructions)
        new = []
        changed = False
        for ins in insts:
            si = ins.sync_info
            waits = list(si.on_wait) if (si and si.on_wait) else []
            if len(waits) > limit:
                for w in waits[limit:]:
                    nop = bass_rust.InstNoOp(
                        name=f"lgl-wait-{n}", ins=[], outs=[]
                    )
                    n += 1
                    nop.engine = ins.engine
                    nop.sync_info = bass_rust.SyncInfo(on_wait=[w], on_update=[])
                    new.append(nop)
                si.on_wait = waits[:limit]
                ins.sync_info = si
                changed = True
            new.append(ins)
        if changed:
            bb.instructions = new
    return n


def _decl(nc, name, shape, dtp):
    return nc.declare_dram_parameter(name, shape, dtp, isOutput=False)


def _build_nc():
    nc = bass.Bass()

    zth = _decl(nc, "zth", [NG, D, TG], BF16)
    ztl = _decl(nc, "ztl", [NG, D, TG], BF16)
    winh = _decl(nc, "winh", [D, P], BF16)
    winl = _decl(nc, "winl", [D, P], BF16)
    w_out = _decl(nc, "w_out", [P, D], BF16)
    eath = _decl(nc, "eath", [P, 448], BF16)   # [emb0|emb1|emb2].T hi
    eatl = _decl(nc, "eatl", [P, 448], BF16)
    embp = {}
    for l, k in enumerate(KS):
        for h in ("h", "l"):
            embp[(l, h)] = _decl(nc, f"emb{l}{h}", [k, P], BF16)
    gp = {}
    for (a, b), shp in [((0, 1), (KS[0], KS[1])), ((0, 2), (KS[0], KS[2])),
                        ((1, 2), (KS[1], KS[2]))]:
        for h in ("h", "l"):
            gp[(a, b, h)] = _decl(nc, f"ng{a}{b}{h}", list(shp), BF16)
    auxp = {}
    for l, k in enumerate(KS):
        for h in ("h", "l"):
            auxp[(l, h)] = _decl(nc, f"aux{l}{h}", [k, 2], BF16)
    b_in = _decl(nc, "b_in", [P, 1], F32)
    b_out = _decl(nc, "b_out", [D, 1], F32)

    out_zq = nc.declare_dram_parameter("out_zq", [NG, D, TG], BF16, isOutput=True)
    out_idx = nc.declare_dram_parameter("out_idx", [NG, 2, 3, TG], F32, isOutput=True)
    out_m = nc.declare_dram_parameter("out_m", [128, NG * NT * 3], F32, isOutput=True)
    out_ssq = nc.declare_dram_parameter("out_ssq", [128, NG * 2], F32, isOutput=True)

    with tile.TileContext(nc) as tc:
        import contextlib

        with contextlib.ExitStack() as ctx:
            singles = ctx.enter_context(tc.tile_pool(name="singles", bufs=1))
            ztp = ctx.enter_context(tc.tile_pool(name="ztp", bufs=2))
            grp = ctx.enter_context(tc.tile_pool(name="grp", bufs=2))
            ohp = ctx.enter_context(tc.tile_pool(name="ohp", bufs=3))
            outp = ctx.enter_context(tc.tile_pool(name="outp", bufs=2))
            psB = ctx.enter_context(tc.tile_pool(name="psB", bufs=4, space="PSUM"))
            psT = ctx.enter_context(tc.tile_pool(name="psT", bufs=2, space="PSUM"))
            psQ = ctx.enter_context(tc.tile_pool(name="psQ", bufs=1, space="PSUM"))

            # ---- resident constants ----
            def load(pool, param, shape, dtp, re=None):
                nm = f"{param.name}_sb"
                t = pool.tile(shape, dtp, name=nm, tag=nm)
                nc.scalar.dma_start(out=t, in_=(param.rearrange(re, p=128) if re else param[:]))
                return t

            winh_sb = load(singles, winh, [128, ND, P], BF16, "(c p) q -> p c q")
            winl_sb = load(singles, winl, [128, ND, P], BF16, "(c p) q -> p c q")
            wout_sb = load(singles, w_out, [128, 2, D], BF16, "(c p) q -> p c q")
            eath_sb = load(singles, eath, [128, 2, 448], BF16, "(c p) q -> p c q")
            eatl_sb = load(singles, eatl, [128, 2, 448], BF16, "(c p) q -> p c q")
            emb_sb = {}
            for l, k in enumerate(KS):
                for h in ("h", "l"):
                    if k <= 128:
                        emb_sb[(l, h)] = load(singles, embp[(l, h)], [k, P], BF16)
                    else:
                        emb_sb[(l, h)] = load(
                            singles, embp[(l, h)], [128, 2, P], BF16, "(c p) q -> p c q"
                        )
            g_sb = {k: load(singles, v, list(v.shape), BF16) for k, v in gp.items()}
            aux_sb = {}
            for l, k in enumerate(KS):
                if k <= 128:
                    aux_sb[l] = load(singles, auxp[(l, "h")], [k, 2], BF16)
                else:
                    aux_sb[l] = load(
                        singles, auxp[(l, "h")], [128, 2, 2], BF16, "(c p) q -> p c q"
                    )
            ident = singles.tile([128, 128], BF16)
            from concourse.masks import make_identity
            make_identity(nc, ident)
            bin_sb = load(singles, b_in, [128, 2], F32, "(c p) q -> p (c q)")
            bout_sb = load(singles, b_out, [128, ND], F32, "(c p) q -> p (c q)")
            m_sb = singles.tile([128, NG * NT * 3], F32)
            ssq_sb = singles.tile([128, NG * 2], F32)
            dummy_sb = singles.tile([128, TG], F32)

            # ---- group-skewed emission: stage A of group g+1 is
            # interleaved into the VQ cascade of group g so the PE always has
            # dense independent matmul work (keeps the HAM clock at 8/8) ----
            st = {}
            A_PASSES = (("h", "h"), ("l", "h"), ("h", "l"))  # (z half, W half)

            def dma_group(g, nchunks=1):
                zh_t = ztp.tile([128, ND, TG], BF16, name=f"zth{g}", tag="zth_sb")
                zl_t = ztp.tile([128, ND, TG], BF16, name=f"ztl{g}", tag="ztl_sb")
                hs = zth[g].rearrange("(c p) t -> p c t", p=128)
                ls = ztl[g].rearrange("(c p) t -> p c t", p=128)
                step = (ND + nchunks - 1) // nchunks
                for c0 in range(0, ND, step):
                    c1 = min(c0 + step, ND)
                    nc.sync.dma_start(out=zh_t[:, c0:c1, :], in_=hs[:, c0:c1, :])
                    nc.sync.dma_start(out=zl_t[:, c0:c1, :], in_=ls[:, c0:c1, :])
                st[g] = {"zth": zh_t, "ztl": zl_t}

            def stageA_alloc(g):
                s = st[g]
                s["r0f"] = grp.tile([128, 2, TG], F32, name=f"r0f{g}", tag="r0f")
                s["r0h"] = grp.tile([128, 2, TG], BF16, name=f"r0h{g}", tag="r0h")
                s["r0l"] = grp.tile([128, 2, TG], BF16, name=f"r0l{g}", tag="r0l")
                s["aops"] = [
                    (pc, di, zs, ws)
                    for pc in range(2)
                    for zs, ws in A_PASSES
                    for di in range(ND)
                ]
                s["aps"] = None

            def _stageA_close_pc(s, g, pc):
                ps = s["aps"]
                nc.scalar.activation(
                    out=s["r0f"][:, pc, :], in_=ps, func=AF.Identity,
                    bias=bin_sb[:, pc : pc + 1],
                )
                nc.scalar.activation(
                    out=s["r0h"][:, pc, :], in_=ps, func=AF.Identity,
                    bias=bin_sb[:, pc : pc + 1],
                )
                nc.gpsimd.tensor_tensor(
                    out=s["r0l"][:, pc, :], in0=s["r0f"][:, pc, :],
                    in1=s["r0h"][:, pc, :], op=ALU.subtract,
                )
                nc.scalar.activation(
                    out=dummy_sb, in_=s["r0f"][:, pc, :], func=AF.Square,
                    accum_out=ssq_sb[:, g * 2 + pc : g * 2 + pc + 1],
                )
                s["aps"] = None

            def stageA_mms(g, _pc_ignored, count):
                s = st[g]
                lst = s["aops"]
                npc = ND * len(A_PASSES)
                for _ in range(min(count, len(lst))):
                    pc, di, zs, ws = lst.pop(0)
                    if s["aps"] is None:
                        s["aps"] = psB.tile(
                            [128, TG], F32, name=f"aps{g}_{pc}", tag="big"
                        )
                    lh = winh_sb if ws == "h" else winl_sb
                    rh = s["zth"] if zs == "h" else s["ztl"]
                    rem_in_pc = sum(1 for x in lst if x[0] == pc)
                    nc.tensor.matmul(
                        s["aps"], lhsT=lh[:, di, ts(pc, 128)], rhs=rh[:, di, :],
                        start=(rem_in_pc == npc - 1), stop=(rem_in_pc == 0),
                    )
                    if rem_in_pc == 0:
                        _stageA_close_pc(s, g, pc)

            def stageA_finish(g):
                stageA_mms(g, 0, 999)

            def pan_alloc(g):
                s = st[g]
                s["pn"] = [
                    grp.tile([64 if w == 0 else 128, NT, 128], BF16,
                             name=f"pn{w}_{g}", tag=f"pn{w}")
                    for w in range(4)
                ]

            def cascade_tile(g, i):
                s = st[g]
                r0h, r0l = s["r0h"], s["r0l"]
                pn0, pn1, pn2, pn3 = s["pn"]
                tsl = ts(i, 128)
                score = psB.tile([128, TG], F32, name=f"score{g}_{i}", tag="big")
                s.setdefault("score", []).append(score)
                for j, (lh, rh) in enumerate(
                    [(r0h, eath_sb), (r0h, eatl_sb), (r0l, eath_sb)]
                ):
                    for pc in range(2):
                        nc.tensor.matmul(
                            score[:, 64:512], lhsT=lh[:, pc, tsl],
                            rhs=rh[:, pc, :],
                            start=(j == 0 and pc == 0), stop=False,
                        )
                oh = ohp.tile([128, TG], BF16, name=f"oh{g}_{i}", tag="oh")
                for lvl in range(3):
                    lo, hi = LOFF[lvl], LEND[lvl]
                    mcol = g * NT * 3 + i * 3 + lvl
                    nc.vector.tensor_reduce(
                        out=m_sb[:, mcol : mcol + 1], in_=score[:, lo:hi],
                        axis=AX.X, op=ALU.max,
                    )
                    nc.vector.tensor_scalar(
                        out=oh[:, lo:hi], in0=score[:, lo:hi],
                        scalar1=m_sb[:, mcol : mcol + 1], scalar2=None,
                        op0=ALU.is_equal,
                    )
                    kk = hi - lo
                    trp = psT.tile([128, 256], BF16, name="trp", tag="trp")
                    if lvl < 2:
                        nc.tensor.transpose(trp[0:kk, 0:128], oh[:, lo:hi], ident)
                        if lvl == 0:
                            nc.scalar.copy(out=pn0[:, i, :], in_=trp[0:kk, 0:128])
                        else:
                            nc.vector.tensor_copy(pn1[:, i, :], trp[:, 0:128])
                    else:
                        for kc in range(2):
                            nc.tensor.transpose(
                                trp[:, kc * 128 : (kc + 1) * 128],
                                oh[:, lo + kc * 128 : lo + (kc + 1) * 128],
                                ident,
                            )
                        nc.scalar.copy(out=pn2[:, i, :], in_=trp[:, 0:128])
                        nc.vector.tensor_copy(pn3[:, i, :], trp[:, 128:256])
                    if lvl == 0:
                        lhs0 = pn0[:, i, :]
                        for h in ("h", "l"):
                            nc.tensor.matmul(
                                score[:, LOFF[1] : LEND[1]], lhsT=lhs0,
                                rhs=g_sb[(0, 1, h)], start=False, stop=False,
                            )
                        for h in ("h", "l"):
                            nc.tensor.matmul(
                                score[:, LOFF[2] : LEND[2]], lhsT=lhs0,
                                rhs=g_sb[(0, 2, h)], start=False, stop=False,
                            )
                    elif lvl == 1:
                        for h in ("h", "l"):
                            nc.tensor.matmul(
                                score[:, LOFF[2] : LEND[2]], lhsT=pn1[:, i, :],
                                rhs=g_sb[(1, 2, h)], start=False, stop=(h == "l"),
                            )

            def gather_aux(g):
                s = st[g]
                pn0, pn1, pn2, pn3 = s["pn"]
                ohT0, ohT1 = pn0[:, :, :], pn1[:, :, :]
                ohT2 = (pn2, pn3)
                zq_ps = psQ.tile([128, 2, TG], F32, name=f"zqps{g}", tag="zqps")
                for pc in range(2):
                    mm = []
                    for h in ("h",):
                        mm.append((emb_sb[(0, h)][:, ts(pc, 128)], ohT0))
                        mm.append((emb_sb[(1, h)][:, ts(pc, 128)], ohT1))
                        for kc in range(2):
                            mm.append(
                                (emb_sb[(2, h)][:, kc, ts(pc, 128)], ohT2[kc][:, :, :])
                            )
                    for j, (lh, rh) in enumerate(mm):
                        nc.tensor.matmul(
                            zq_ps[:, pc, :], lhsT=lh, rhs=rh,
                            start=(j == 0), stop=(j == len(mm) - 1),
                        )
                zq_sb = grp.tile([128, 2, TG], BF16, name=f"zq{g}", tag="zq_sb")
                s["zq"] = zq_sb
                for pc in range(2):
                    nc.scalar.copy(out=zq_sb[:, pc, :], in_=zq_ps[:, pc, :])

                axo = grp.tile([2, 3, TG], F32, name=f"axo{g}", tag="axo")
                for lvl in range(3):
                    xps = psB.tile([2, TG], F32, name=f"xps{g}_{lvl}", tag="big")
                    mm = []
                    if lvl == 0:
                        mm.append((aux_sb[0], ohT0))
                    elif lvl == 1:
                        mm.append((aux_sb[1], ohT1))
                    else:
                        for kc in range(2):
                            mm.append((aux_sb[2][:, kc, :], ohT2[kc][:, :, :]))
                    for j, (lh, rh) in enumerate(mm):
                        nc.tensor.matmul(
                            xps, lhsT=lh, rhs=rh,
                            start=(j == 0), stop=(j == len(mm) - 1),
                        )
                    nc.vector.tensor_copy(axo[:, lvl, :], xps)
                nc.sync.dma_start(out=out_idx[g], in_=axo)

            def stageD_chunk(g, i):
                s = st[g]
                zq_sb = s["zq"]
                if "outT" not in s:
                    s["outT"] = outp.tile([128, ND, TG], BF16, name=f"outT{g}", tag="outT")
                    s["ddone"] = 0
                outT_sb = s["outT"]
                ozq = out_zq[g].rearrange("(c p) t -> p c t", p=128)
                done = s["ddone"]
                for di in range(3 * i, min(3 * i + 3, ND)):
                    ps = psB.tile([128, TG], F32, name=f"dps{g}_{di}", tag="big")
                    for pc in range(2):
                        nc.tensor.matmul(
                            ps, lhsT=wout_sb[:, pc, ts(di, 128)],
                            rhs=zq_sb[:, pc, :],
                            start=(pc == 0), stop=(pc == 1),
                        )
                    if di % 2 == 0:
                        nc.scalar.activation(
                            out=outT_sb[:, di, :], in_=ps, func=AF.Identity,
                            bias=bout_sb[:, di : di + 1],
                        )
                    else:
                        nc.vector.tensor_scalar_add(
                            out=outT_sb[:, di, :], in0=ps,
                            scalar1=bout_sb[:, di : di + 1],
                        )
                    if di + 1 - done >= 3 or di == ND - 1:
                        nc.sync.dma_start(
                            out=ozq[:, done : di + 1, :],
                            in_=outT_sb[:, done : di + 1, :],
                        )
                        done = di + 1
                s["ddone"] = done

            dma_group(0)
            stageA_alloc(0)
            stageA_finish(0)
            dma_group(1)
            stageA_alloc(1)
            stageA_mms(1, 0, 22)
            for g in range(NG):
                pan_alloc(g)
                if g + 1 < NG and "r0f" not in st[g + 1]:
                    stageA_alloc(g + 1)
                for i in range(NT):
                    cascade_tile(g, i)
                    if g > 0:
                        stageD_chunk(g - 1, i)
                    if g + 1 < NG:
                        stageA_mms(g + 1, 0, 16)
                gather_aux(g)
                if g + 1 < NG:
                    stageA_finish(g + 1)
                if g + 2 < NG:
                    dma_group(g + 2)
            for i in range(NT):
                stageD_chunk(NG - 1, i)

            nc.sync.dma_start(out=out_m[:], in_=m_sb)
            nc.sync.dma_start(out=out_ssq[:], in_=ssq_sb)

    _legalize_sync_waits(nc)
    return nc


_NC_CACHE = None


def _get_nc():
    global _NC_CACHE
    if _NC_CACHE is None:
        _NC_CACHE = _build_nc()
    return _NC_CACHE


def _fp22(x):
    return (np.asarray(x, np.float32).view(np.int32) & np.int32(~0x3FF)).view(
        np.float32
    )


def _pair(x):
    x = np.asarray(x, np.float32)
    h = x.astype(NPBF)
    l = (x - h.astype(np.float32)).astype(NPBF)
    return np.ascontiguousarray(h), np.ascontiguousarray(l)


def kernel(z, W_in, b_in, W_out, b_out, emb0, emb1, emb2, _trace=False):
    z = np.asarray(z, np.float32)
    W_in = np.asarray(W_in, np.float32)
    b_in = np.asarray(b_in, np.float32)
    W_out = np.asarray(W_out, np.float32)
    b_out = np.asarray(b_out, np.float32)
    embs = [np.asarray(e, np.float32) for e in (emb0, emb1, emb2)]

    e64 = [e.astype(np.float64) for e in embs]
    shared = {}
    shared["winh"], shared["winl"] = _pair(W_in)
    shared["w_out"] = np.ascontiguousarray(W_out.astype(NPBF))
    eat = np.concatenate([e.T for e in embs], axis=1)  # [256, 448]
    shared["eath"], shared["eatl"] = _pair(eat)
    for l in range(3):
        shared[f"emb{l}h"], shared[f"emb{l}l"] = _pair(embs[l])
    for (a, b), nm in [((0, 1), "ng01"), ((0, 2), "ng02"), ((1, 2), "ng12")]:
        g = -(e64[a] @ e64[b].T)
        shared[nm + "h"], shared[nm + "l"] = _pair(g)
    e2 = [np.sum(e * e, axis=1) for e in e64]
    for l, k in enumerate(KS):
        a = np.stack([np.arange(k, dtype=np.float64), e2[l]], axis=1)
        shared[f"aux{l}h"], shared[f"aux{l}l"] = _pair(a)
    shared["b_in"] = np.ascontiguousarray(b_in.reshape(P, 1))
    shared["b_out"] = np.ascontiguousarray(b_out.reshape(D, 1))

    bpc = B // NCORES
    zf = z.reshape(NCORES, TOK, D)
    in_maps = []
    for c in range(NCORES):
        zh, zl = _pair(zf[c])
        in_maps.append(
            {
                "zth": np.ascontiguousarray(zh.T.reshape(D, NG, TG).transpose(1, 0, 2)),
                "ztl": np.ascontiguousarray(zl.T.reshape(D, NG, TG).transpose(1, 0, 2)),
                **shared,
            }
        )

    nc = _get_nc()
    last_err = None
    for _attempt in range(3):
        try:
            res = run_bass_kernel_spmd(
                nc, in_maps, core_ids=list(range(NCORES)), trace=_trace
            )
            break
        except Exception as e:  # transient NRT/axon device hiccups
            last_err = e
            import time as _time

            _time.sleep(2.0)
    else:
        raise last_err

    zq_full = np.empty((B, N, D), np.float32)
    idxs = [np.empty((B, N), np.int32) for _ in range(3)]
    S0 = 0.0
    M = np.zeros(3, np.float64)
    E = np.zeros(3, np.float64)
    for c in range(NCORES):
        r = res.results[c]
        zqb = np.asarray(r["out_zq"], np.float32)  # [NG, D, TG] (bf16 on device)
        zq_full[c * bpc : (c + 1) * bpc] = (
            zqb.transpose(1, 0, 2).reshape(D, TOK).T.reshape(bpc, N, D)
        )
        ax = r["out_idx"]  # [NG, 2, 3, TG]
        for l in range(3):
            idxs[l][c * bpc : (c + 1) * bpc] = (
                np.rint(ax[:, 0, l, :]).astype(np.int32).reshape(bpc, N)
            )
            E[l] += ax[:, 1, l, :].astype(np.float64).sum()
        S0 += r["out_ssq"].astype(np.float64).sum()
        mst = r["out_m"].astype(np.float64).reshape(128, NG * NT, 3)
        for l in range(3):
            M[l] += mst[:, :, l].sum()

    S1 = S0 - 2.0 * M[0] + E[0]
    S2 = S1 - 2.0 * M[1] + E[1]
    S3 = S2 - 2.0 * M[2] + E[2]
    vq_loss = np.float32(COMMIT * (S1 + S2 + S3) / (3.0 * B * N * P))

    out = (zq_full, idxs[0], idxs[1], idxs[2], vq_loss)
    if _trace:
        return out, res
    return out


# revision 32
# speedup vs baseline: 1.0072x; 1.0072x over previous
"""Trainium2 Bass kernel for AIMQuantizerForVJEPA (residual VQ, 3 levels).

Math (forward pass):
  r0 = z @ W_in + b_in                      [BN, 256]
  per level l: score_l = r_l @ emb_l.T ; idx_l = argmax(score_l)
  (equivalent to argmin of squared distance since ||e_k||^2 == 1 up to ~1e-7)
  z_q_l = emb_l[idx_l] ; r_{l+1} = r_l - z_q_l
  z_q_out = (z_q_0+z_q_1+z_q_2) @ W_out + b_out
  vq_loss = 0.25/(3*BN*P) * (S1+S2+S3),  S_{l+1} = sum_t ||r_{l+1},t||^2
  with the telescoping identity S_{l+1} = S_l - 2*sum_t max_t + sum_t ||e_sel||^2
  evaluated on the host from small device-side stashes.

Precision: all argmax-relevant matmuls run as bf16 3-term splits
(ah@bh + ah@bl + al@bh with x = xh + xl, xh=bf16(x), xl=bf16(x-xh));
operand error ~2^-17, products accumulate in fp32 -> fp32-grade scores at
3 cycles/row instead of fp32's 4, with FWL-fast weight loads.
Residuals for levels 1/2 are never materialized: scores are corrected with
host-precomputed Gram matrices (score_1 = r0@emb1.T - G01[idx0], ...), the
row gathers being onehot matmuls. Onehot transposes ride the DMA xbar
(bf16-only path), not the PE. The output-side matmuls (onehot@emb, zq@W_out)
use float32r / bf16 pairs; their error does not feed any argmax.

Sharding: data-parallel over batch, 2 batches (4096 tokens) per core, 8 cores.
Device layout is channel-on-partitions; the host pre/post-transposes z and
z_q_out.
"""

import sys

for _p in ("/opt/trn_rl_repo", "/opt/pypackages"):
    if _p not in sys.path:
        sys.path.insert(0, _p)

import ml_dtypes
import numpy as np

import concourse.bass as bass
import concourse.mybir as mybir
import concourse.tile as tile
from concourse.bass import ts
from concourse.bass_utils import run_bass_kernel_spmd

dt = mybir.dt
F32 = dt.float32
F32R = dt.float32r
BF16 = dt.bfloat16
AF = mybir.ActivationFunctionType
ALU = mybir.AluOpType
AX = mybir.AxisListType
NPBF = ml_dtypes.bfloat16

B, N, D, P = 16, 2048, 1408, 256
KS = (64, 128, 256)
COMMIT = 0.25
NCORES = 8
TOK = B * N // NCORES        # 4096 tokens per core
NG = 8                       # groups per core
TG = TOK // NG               # 512 tokens per group
NT = TG // 128               # 4 tiles of 128 tokens per group
ND = D // 128                # 11 d-tiles
# level column ranges inside the 512-wide score panel (128-aligned windows so
# the DMA-xbar transpose of each window lands as one [128,128] block)
LOFF = (64, 128, 256)
LEND = (128, 256, 512)


def _legalize_sync_waits(nc, limit=1):
    """This walrus build allows only one sync-wait per instruction; move
    excess waits onto dedicated NoOps just before their instruction."""
    import bass_rust

    n = 0
    for bb in nc.main_func.blocks:
        insts = list(bb.instructions)
        new = []
        changed = False
        for ins in insts:
            si = ins.sync_info
            waits = list(si.on_wait) if (si and si.on_wait) else []
            if len(waits) > limit:
                for w in waits[limit:]:
                    nop = bass_rust.InstNoOp(
                        name=f"lgl-wait-{n}", ins=[], outs=[]
                    )
                    n += 1
                    nop.engine = ins.engine
                    nop.sync_info = bass_rust.SyncInfo(on_wait=[w], on_update=[])
                    new.append(nop)
                si.on_wait = waits[:limit]
                ins.sync_info = si
                changed = True
            new.append(ins)
        if changed:
            bb.instructions = new
    return n


def _decl(nc, name, shape, dtp):
    return nc.declare_dram_parameter(name, shape, dtp, isOutput=False)


def _build_nc():
    nc = bass.Bass()

    zth = _decl(nc, "zth", [NG, D, TG], BF16)
    ztl = _decl(nc, "ztl", [NG, D, TG], BF16)
    winh = _decl(nc, "winh", [D, P], BF16)
    winl = _decl(nc, "winl", [D, P], BF16)
    w_out = _decl(nc, "w_out", [P, D], BF16)
    eath = _decl(nc, "eath", [P, 448], BF16)   # [emb0|emb1|emb2].T hi
    eatl = _decl(nc, "eatl", [P, 448], BF16)
    embp = {}
    for l, k in enumerate(KS):
        for h in ("h", "l"):
            embp[(l, h)] = _decl(nc, f"emb{l}{h}", [k, P], BF16)
    gp = {}
    for (a, b), shp in [((0, 1), (KS[0], KS[1])), ((0, 2), (KS[0], KS[2])),
                        ((1, 2), (KS[1], KS[2]))]:
        for h in ("h", "l"):
            gp[(a, b, h)] = _decl(nc, f"ng{a}{b}{h}", list(shp), BF16)
    auxp = {}
    for l, k in enumerate(KS):
        for h in ("h", "l"):
            auxp[(l, h)] = _decl(nc, f"aux{l}{h}", [k, 2], BF16)
    b_in = _decl(nc, "b_in", [P, 1], F32)
    b_out = _decl(nc, "b_out", [D, 1], F32)

    out_zq = nc.declare_dram_parameter("out_zq", [NG, D, TG], BF16, isOutput=True)
    out_idx = nc.declare_dram_parameter("out_idx", [NG, 2, 3, TG], F32, isOutput=True)
    out_m = nc.declare_dram_parameter("out_m", [128, NG * NT * 3], F32, isOutput=True)
    out_ssq = nc.declare_dram_parameter("out_ssq", [128, NG * 2], F32, isOutput=True)

    with tile.TileContext(nc) as tc:
        import contextlib

        with contextlib.ExitStack() as ctx:
            singles = ctx.enter_context(tc.tile_pool(name="singles", bufs=1))
            ztp = ctx.enter_context(tc.tile_pool(name="ztp", bufs=2))
            grp = ctx.enter_context(tc.tile_pool(name="grp", bufs=2))
            ohp = ctx.enter_context(tc.tile_pool(name="ohp", bufs=3))
            outp = ctx.enter_context(tc.tile_pool(name="outp", bufs=2))
            psB = ctx.enter_context(tc.tile_pool(name="psB", bufs=4, space="PSUM"))
            psT = ctx.enter_context(tc.tile_pool(name="psT", bufs=2, space="PSUM"))
            psQ = ctx.enter_context(tc.tile_pool(name="psQ", bufs=1, space="PSUM"))

            # ---- resident constants ----
            def load(pool, param, shape, dtp, re=None):
                nm = f"{param.name}_sb"
                t = pool.tile(shape, dtp, name=nm, tag=nm)
                nc.scalar.dma_start(out=t, in_=(param.rearrange(re, p=128) if re else param[:]))
                return t

            winh_sb = load(singles, winh, [128, ND, P], BF16, "(c p) q -> p c q")
            winl_sb = load(singles, winl, [128, ND, P], BF16, "(c p) q -> p c q")
            wout_sb = load(singles, w_out, [128, 2, D], BF16, "(c p) q -> p c q")
            eath_sb = load(singles, eath, [128, 2, 448], BF16, "(c p) q -> p c q")
            eatl_sb = load(singles, eatl, [128, 2, 448], BF16, "(c p) q -> p c q")
            emb_sb = {}
            for l, k in enumerate(KS):
                for h in ("h", "l"):
                    if k <= 128:
                        emb_sb[(l, h)] = load(singles, embp[(l, h)], [k, P], BF16)
                    else:
                        emb_sb[(l, h)] = load(
                            singles, embp[(l, h)], [128, 2, P], BF16, "(c p) q -> p c q"
                        )
            g_sb = {k: load(singles, v, list(v.shape), BF16) for k, v in gp.items()}
            aux_sb = {}
            for l, k in enumerate(KS):
                if k <= 128:
                    aux_sb[l] = load(singles, auxp[(l, "h")], [k, 2], BF16)
                else:
                    aux_sb[l] = load(
                        singles, auxp[(l, "h")], [128, 2, 2], BF16, "(c p) q -> p c q"
                    )
            ident = singles.tile([128, 128], BF16)
            from concourse.masks import make_identity
            make_identity(nc, ident)
            bin_sb = load(singles, b_in, [128, 2], F32, "(c p) q -> p (c q)")
            bout_sb = load(singles, b_out, [128, ND], F32, "(c p) q -> p (c q)")
            m_sb = singles.tile([128, NG * NT * 3], F32)
            ssq_sb = singles.tile([128, NG * 2], F32)
            dummy_sb = singles.tile([128, TG], F32)

            # ---- group-skewed emission: stage A of group g+1 is
            # interleaved into the VQ cascade of group g so the PE always has
            # dense independent matmul work (keeps the HAM clock at 8/8) ----
            st = {}
            A_PASSES = (("h", "h"), ("l", "h"), ("h", "l"))  # (z half, W half)

            def dma_group(g, nchunks=1):
                zh_t = ztp.tile([128, ND, TG], BF16, name=f"zth{g}", tag="zth_sb")
                zl_t = ztp.tile([128, ND, TG], BF16, name=f"ztl{g}", tag="ztl_sb")
                hs = zth[g].rearrange("(c p) t -> p c t", p=128)
                ls = ztl[g].rearrange("(c p) t -> p c t", p=128)
                step = (ND + nchunks - 1) // nchunks
                for c0 in range(0, ND, step):
                    c1 = min(c0 + step, ND)
                    nc.sync.dma_start(out=zh_t[:, c0:c1, :], in_=hs[:, c0:c1, :])
                    nc.sync.dma_start(out=zl_t[:, c0:c1, :], in_=ls[:, c0:c1, :])
                st[g] = {"zth": zh_t, "ztl": zl_t}

            def stageA_alloc(g):
                s = st[g]
                s["r0f"] = grp.tile([128, 2, TG], F32, name=f"r0f{g}", tag="r0f")
                s["r0h"] = grp.tile([128, 2, TG], BF16, name=f"r0h{g}", tag="r0h")
                s["r0l"] = grp.tile([128, 2, TG], BF16, name=f"r0l{g}", tag="r0l")
                s["aops"] = [
                    (pc, di, zs, ws)
                    for pc in range(2)
                    for zs, ws in A_PASSES
                    for di in range(ND)
                ]
                s["aps"] = None

            def _stageA_close_pc(s, g, pc):
                ps = s["aps"]
                nc.scalar.activation(
                    out=s["r0f"][:, pc, :], in_=ps, func=AF.Identity,
                    bias=bin_sb[:, pc : pc + 1],
                )
                nc.scalar.activation(
                    out=s["r0h"][:, pc, :], in_=ps, func=AF.Identity,
                    bias=bin_sb[:, pc : pc + 1],
                )
                nc.gpsimd.tensor_tensor(
                    out=s["r0l"][:, pc, :], in0=s["r0f"][:, pc, :],
                    in1=s["r0h"][:, pc, :], op=ALU.subtract,
                )
                nc.scalar.activation(
                    out=dummy_sb, in_=s["r0f"][:, pc, :], func=AF.Square,
                    accum_out=ssq_sb[:, g * 2 + pc : g * 2 + pc + 1],
                )
                s["aps"] = None

            def stageA_mms(g, _pc_ignored, count):
                s = st[g]
                lst = s["aops"]
                npc = ND * len(A_PASSES)
                for _ in range(min(count, len(lst))):
                    pc, di, zs, ws = lst.pop(0)
                    if s["aps"] is None:
                        s["aps"] = psB.tile(
                            [128, TG], F32, name=f"aps{g}_{pc}", tag="big"
                        )
                    lh = winh_sb if ws == "h" else winl_sb
                    rh = s["zth"] if zs == "h" else s["ztl"]
                    rem_in_pc = sum(1 for x in lst if x[0] == pc)
                    nc.tensor.matmul(
                        s["aps"], lhsT=lh[:, di, ts(pc, 128)], rhs=rh[:, di, :],
                        start=(rem_in_pc == npc - 1), stop=(rem_in_pc == 0),
                    )
                    if rem_in_pc == 0:
                        _stageA_close_pc(s, g, pc)

            def stageA_finish(g):
                stageA_mms(g, 0, 999)

            def pan_alloc(g):
                s = st[g]
                s["pn"] = [
                    grp.tile([64 if w == 0 else 128, NT, 128], BF16,
                             name=f"pn{w}_{g}", tag=f"pn{w}")
                    for w in range(4)
                ]

            def cascade_tile(g, i):
                s = st[g]
                r0h, r0l = s["r0h"], s["r0l"]
                pn0, pn1, pn2, pn3 = s["pn"]
                tsl = ts(i, 128)
                score = psB.tile([128, TG], F32, name=f"score{g}_{i}", tag="big")
                s.setdefault("score", []).append(score)
                for j, (lh, rh) in enumerate(
                    [(r0h, eath_sb), (r0h, eatl_sb), (r0l, eath_sb)]
                ):
                    for pc in range(2):
                        nc.tensor.matmul(
                            score[:, 64:512], lhsT=lh[:, pc, tsl],
                            rhs=rh[:, pc, :],
                            start=(j == 0 and pc == 0), stop=False,
                        )
                oh = ohp.tile([128, TG], BF16, name=f"oh{g}_{i}", tag="oh")
                for lvl in range(3):
                    lo, hi = LOFF[lvl], LEND[lvl]
                    mcol = g * NT * 3 + i * 3 + lvl
                    nc.vector.tensor_reduce(
                        out=m_sb[:, mcol : mcol + 1], in_=score[:, lo:hi],
                        axis=AX.X, op=ALU.max,
                    )
                    nc.vector.tensor_scalar(
                        out=oh[:, lo:hi], in0=score[:, lo:hi],
                        scalar1=m_sb[:, mcol : mcol + 1], scalar2=None,
                        op0=ALU.is_equal,
                    )
                    kk = hi - lo
                    trp = psT.tile([128, 256], BF16, name="trp", tag="trp")
                    if lvl < 2:
                        nc.tensor.transpose(trp[0:kk, 0:128], oh[:, lo:hi], ident)
                        if lvl == 0:
                            nc.scalar.copy(out=pn0[:, i, :], in_=trp[0:kk, 0:128])
                        else:
                            nc.vector.tensor_copy(pn1[:, i, :], trp[:, 0:128])
                    else:
                        for kc in range(2):
                            nc.tensor.transpose(
                                trp[:, kc * 128 : (kc + 1) * 128],
                                oh[:, lo + kc * 128 : lo + (kc + 1) * 128],
                                ident,
                            )
                        nc.scalar.copy(out=pn2[:, i, :], in_=trp[:, 0:128])
                        nc.vector.tensor_copy(pn3[:, i, :], trp[:, 128:256])
                    if lvl == 0:
                        lhs0 = pn0[:, i, :]
                        for h in ("h", "l"):
                            nc.tensor.matmul(
                                score[:, LOFF[1] : LEND[1]], lhsT=lhs0,
                                rhs=g_sb[(0, 1, h)], start=False, stop=False,
                            )
                        for h in ("h", "l"):
                            nc.tensor.matmul(
                                score[:, LOFF[2] : LEND[2]], lhsT=lhs0,
                                rhs=g_sb[(0, 2, h)], start=False, stop=False,
                            )
                    elif lvl == 1:
                        for h in ("h", "l"):
                            nc.tensor.matmul(
                                score[:, LOFF[2] : LEND[2]], lhsT=pn1[:, i, :],
                                rhs=g_sb[(1, 2, h)], start=False, stop=(h == "l"),
                            )

            def gather_aux(g):
                s = st[g]
                pn0, pn1, pn2, pn3 = s["pn"]
                ohT0, ohT1 = pn0[:, :, :], pn1[:, :, :]
                ohT2 = (pn2, pn3)
                zq_ps = psQ.tile([128, 2, TG], F32, name=f"zqps{g}", tag="zqps")
                for pc in range(2):
                    mm = []
                    for h in ("h",):
                        mm.append((emb_sb[(0, h)][:, ts(pc, 128)], ohT0))
                        mm.append((emb_sb[(1, h)][:, ts(pc, 128)], ohT1))
                        for kc in range(2):
                            mm.append(
                                (emb_sb[(2, h)][:, kc, ts(pc, 128)], ohT2[kc][:, :, :])
                            )
                    for j, (lh, rh) in enumerate(mm):
                        nc.tensor.matmul(
                            zq_ps[:, pc, :], lhsT=lh, rhs=rh,
                            start=(j == 0), stop=(j == len(mm) - 1),
                        )
                zq_sb = grp.tile([128, 2, TG], BF16, name=f"zq{g}", tag="zq_sb")
                s["zq"] = zq_sb
                for pc in range(2):
                    nc.scalar.copy(out=zq_sb[:, pc, :], in_=zq_ps[:, pc, :])

                axo = grp.tile([2, 3, TG], F32, name=f"axo{g}", tag="axo")
                for lvl in range(3):
                    xps = psB.tile([2, TG], F32, name=f"xps{g}_{lvl}", tag="big")
                    mm = []
                    if lvl == 0:
                        mm.append((aux_sb[0], ohT0))
                    elif lvl == 1:
                        mm.append((aux_sb[1], ohT1))
                    else:
                        for kc in range(2):
                            mm.append((aux_sb[2][:, kc, :], ohT2[kc][:, :, :]))
                    for j, (lh, rh) in enumerate(mm):
                        nc.tensor.matmul(
                            xps, lhsT=lh, rhs=rh,
                            start=(j == 0), stop=(j == len(mm) - 1),
                        )
                    nc.vector.tensor_copy(axo[:, lvl, :], xps)
                nc.sync.dma_start(out=out_idx[g], in_=axo)

            def stageD_chunk(g, i):
                s = st[g]
                zq_sb = s["zq"]
                if "outT" not in s:
                    s["outT"] = outp.tile([128, ND, TG], BF16, name=f"outT{g}", tag="outT")
                    s["ddone"] = 0
                outT_sb = s["outT"]
                ozq = out_zq[g].rearrange("(c p) t -> p c t", p=128)
                done = s["ddone"]
                for di in range(3 * i, min(3 * i + 3, ND)):
                    ps = psB.tile([128, TG], F32, name=f"dps{g}_{di}", tag="big")
                    for pc in range(2):
                        nc.tensor.matmul(
                            ps, lhsT=wout_sb[:, pc, ts(di, 128)],
                            rhs=zq_sb[:, pc, :],
                            start=(pc == 0), stop=(pc == 1),
                        )
                    if di % 2 == 0:
                        nc.scalar.activation(
                            out=outT_sb[:, di, :], in_=ps, func=AF.Identity,
                            bias=bout_sb[:, di : di + 1],
                        )
                    else:
                        nc.vector.tensor_scalar_add(
                            out=outT_sb[:, di, :], in0=ps,
                            scalar1=bout_sb[:, di : di + 1],
                        )
                    if di + 1 - done >= 3 or di == ND - 1:
                        nc.sync.dma_start(
                            out=ozq[:, done : di + 1, :],
                            in_=outT_sb[:, done : di + 1, :],
                        )
                        done = di + 1
                s["ddone"] = done

            dma_group(0)
            stageA_alloc(0)
            stageA_finish(0)
            dma_group(1)
            stageA_alloc(1)
            stageA_mms(1, 0, 22)
            for g in range(NG):
                pan_alloc(g)
                if g + 1 < NG and "r0f" not in st[g + 1]:
                    stageA_alloc(g + 1)
                for i in range(NT):
                    cascade_tile(g, i)
                    if g > 0:
                        stageD_chunk(g - 1, i)
                    if g + 1 < NG:
                        stageA_mms(g + 1, 0, 16)
                gather_aux(g)
                if g + 1 < NG:
                    stageA_finish(g + 1)
                if g + 2 < NG:
                    dma_group(g + 2)
            for i in range(NT):
                stageD_chunk(NG - 1, i)

            nc.sync.dma_start(out=out_m[:], in_=m_sb)
            nc.sync.dma_start(out=out_ssq[:], in_=ssq_sb)

    _legalize_sync_waits(nc)
    return nc


_NC_CACHE = None


def _get_nc():
    global _NC_CACHE
    if _NC_CACHE is None:
        _NC_CACHE = _build_nc()
    return _NC_CACHE


def _fp22(x):
    return (np.asarray(x, np.float32).view(np.int32) & np.int32(~0x3FF)).view(
        np.float32
    )


def _pair(x):
    x = np.asarray(x, np.float32)
    h = x.astype(NPBF)
    l = (x - h.astype(np.float32)).astype(NPBF)
    return np.ascontiguousarray(h), np.ascontiguousarray(l)


def kernel(z, W_in, b_in, W_out, b_out, emb0, emb1, emb2, _trace=False):
    z = np.asarray(z, np.float32)
    W_in = np.asarray(W_in, np.float32)
    b_in = np.asarray(b_in, np.float32)
    W_out = np.asarray(W_out, np.float32)
    b_out = np.asarray(b_out, np.float32)
    embs = [np.asarray(e, np.float32) for e in (emb0, emb1, emb2)]

    e64 = [e.astype(np.float64) for e in embs]
    shared = {}
    shared["winh"], shared["winl"] = _pair(W_in)
    shared["w_out"] = np.ascontiguousarray(W_out.astype(NPBF))
    eat = np.concatenate([e.T for e in embs], axis=1)  # [256, 448]
    shared["eath"], shared["eatl"] = _pair(eat)
    for l in range(3):
        shared[f"emb{l}h"], shared[f"emb{l}l"] = _pair(embs[l])
    for (a, b), nm in [((0, 1), "ng01"), ((0, 2), "ng02"), ((1, 2), "ng12")]:
        g = -(e64[a] @ e64[b].T)
        shared[nm + "h"], shared[nm + "l"] = _pair(g)
    e2 = [np.sum(e * e, axis=1) for e in e64]
    for l, k in enumerate(KS):
        a = np.stack([np.arange(k, dtype=np.float64), e2[l]], axis=1)
        shared[f"aux{l}h"], shared[f"aux{l}l"] = _pair(a)
    shared["b_in"] = np.ascontiguousarray(b_in.reshape(P, 1))
    shared["b_out"] = np.ascontiguousarray(b_out.reshape(D, 1))

    bpc = B // NCORES
    zf = z.reshape(NCORES, TOK, D)
    in_maps = []
    for c in range(NCORES):
        zh, zl = _pair(zf[c])
        in_maps.append(
            {
                "zth": np.ascontiguousarray(zh.T.reshape(D, NG, TG).transpose(1, 0, 2)),
                "ztl": np.ascontiguousarray(zl.T.reshape(D, NG, TG).transpose(1, 0, 2)),
                **shared,
            }
        )

    nc = _get_nc()
    last_err = None
    for _attempt in range(4):
        try:
            res = run_bass_kernel_spmd(
                nc, in_maps, core_ids=list(range(NCORES)), trace=_trace
            )
            break
        except Exception as e:  # transient NRT/axon device hiccups
            last_err = e
            import time as _time

            _time.sleep(5.0 * (_attempt + 1))
            try:  # the PJRT client is poisoned after a device fault;
                # drop it so the next attempt reconnects fresh
                import jax

                jax.clear_caches()
                jax.extend.backend.clear_backends()
            except Exception:
                pass
    else:
        raise last_err

    zq_full = np.empty((B, N, D), np.float32)
    idxs = [np.empty((B, N), np.int32) for _ in range(3)]
    S0 = 0.0
    M = np.zeros(3, np.float64)
    E = np.zeros(3, np.float64)
    for c in range(NCORES):
        r = res.results[c]
        zqb = np.asarray(r["out_zq"], np.float32)  # [NG, D, TG] (bf16 on device)
        zq_full[c * bpc : (c + 1) * bpc] = (
            zqb.transpose(1, 0, 2).reshape(D, TOK).T.reshape(bpc, N, D)
        )
        ax = r["out_idx"]  # [NG, 2, 3, TG]
        for l in range(3):
            idxs[l][c * bpc : (c + 1) * bpc] = (
                np.rint(ax[:, 0, l, :]).astype(np.int32).reshape(bpc, N)
            )
            E[l] += ax[:, 1, l, :].astype(np.float64).sum()
        S0 += r["out_ssq"].astype(np.float64).sum()
        mst = r["out_m"].astype(np.float64).reshape(128, NG * NT, 3)
        for l in range(3):
            M[l] += mst[:, :, l].sum()

    S1 = S0 - 2.0 * M[0] + E[0]
    S2 = S1 - 2.0 * M[1] + E[1]
    S3 = S2 - 2.0 * M[2] + E[2]
    vq_loss = np.float32(COMMIT * (S1 + S2 + S3) / (3.0 * B * N * P))

    out = (zq_full, idxs[0], idxs[1], idxs[2], vq_loss)
    if _trace:
        return out, res
    return out
